# revision 8
# baseline (speedup 1.0000x reference)
"""CrossModalCenterLoss on 8 Trainium2 NeuronCores.

The reference masks the [B, C] distance matrix down to the label-matching
column per row BEFORE clamping, so the loss is exactly

    loss = (sum_b clip(||x_b - centers[labels_b]||^2, 1e-12, 1e12)) / B
         + (C - 1) * 1e-12

No [B, C] matmul is needed — just a gather and a fused squared-distance
reduction. Data-parallel over batch: each of the 8 cores handles 512 rows,
gathers its 512 center rows on-device (centers stay in DRAM, replicated),
computes the per-core partial sum, and the host all-reduces the 8 partials.

v2 layout/schedule (vs the earlier 4x-indirect-DMA version):
  - The Scalar engine's HWDGE ring issues both input DMAs (it exits the
    entry barrier ~1 us before Sync): the gather indices (one small
    [16, 32] int16 tile in the dma_gather wrapped-index layout, built on
    the host) and then x as [128, 4*256] (rows 4p..4p+3 on partition p,
    128 contiguous 4 KiB descriptors).
  - One gpsimd.dma_gather moves all 512 center rows in a single Q7
    instruction (994 ns fixed + 0.34 ns/descriptor) instead of 4 serial
    DMA_INDIRECTs (~1.1 us each). dma_gather lives in the 'mlp' ucode
    library, so the library reload is issued explicitly BEFORE the
    index-DMA wait, overlapping the reload with the DMA flight.
  - DVE computes the whole reduction in two instructions: one
    tensor_tensor subtract over [128, 1024] and one scalar_tensor_tensor
    (d*d with fused row-sum accumulator) — instead of 8 TTs + 3 reduces.
  - PE collapses the [128,1] partials with a const-ones matmul; DVE
    copies PSUM->SBUF (DMA cannot read PSUM); Sync stores the scalar and
    clears semaphores; Scalar parks on the store-ack sem so the NEFF
    cannot complete before the output write is acked.

Raw bacc (no Tile) with manual semaphores; the Bass-constructor
all-engine barrier is skipped (its only consumers are preamble memsets
that retire far before the PE consumes the const-1.0 column).
"""

import numpy as np

_N_CORES = 8
_B = 4096
_D = 256
_C = 10000
_ROWS = _B // _N_CORES  # 512 rows per core
_P = 128
_K = _ROWS // _P  # 4 rows per partition
_CLAMP_MIN = 1e-12

# "gather1": single dma_gather (mlp ucode library). "indirect4": 4 serial
# indirect DMAs on the always-resident SWDGE ucode.
_GATHER_MODE = "indirect4"

_compiled = None


def _build():
    import concourse.bass as bass
    import concourse.mybir as mybir
    from concourse import bacc
    from concourse import library_config

    _orig_barrier = bass.Bass.all_engine_barrier

    def _no_barrier(self, *a, **kw):
        return None

    bass.Bass.all_engine_barrier = _no_barrier
    try:
        nc = bacc.Bacc(
            "TRN2",
            target_bir_lowering=False,
            debug=False,
            num_devices=_N_CORES,
            enable_partition_id=False,
        )
    finally:
        bass.Bass.all_engine_barrier = _orig_barrier

    x = nc.declare_dram_parameter("x", [_ROWS, _D], mybir.dt.float32, isOutput=False)
    centers = nc.declare_dram_parameter(
        "centers", [_C, _D], mybir.dt.float32, isOutput=False
    )
    out = nc.declare_dram_parameter("out", [1, 1], mybir.dt.float32, isOutput=True)
    if _GATHER_MODE == "gather1":
        idx = nc.declare_dram_parameter(
            "idx", [16, _ROWS // 16], mybir.dt.int16, isOutput=False
        )
    else:
        idx = nc.declare_dram_parameter(
            "idx", [_P, _K], mybir.dt.int32, isOutput=False
        )

    F = _K * _D  # 1024 free elements per partition

    ones = nc.const_aps.aps[(mybir.dt.float32, 1.0)]  # [128, 1], preamble-initialized

    from contextlib import ExitStack

    with ExitStack() as ctx:
        if _GATHER_MODE == "gather1":
            lab = ctx.enter_context(
                nc.sbuf_tensor([_P, _ROWS // 16], mybir.dt.int16)
            )
        else:
            lab = ctx.enter_context(nc.sbuf_tensor([_P, _K], mybir.dt.int32))
        xt = ctx.enter_context(nc.sbuf_tensor([_P, F], mybir.dt.float32))
        gt = ctx.enter_context(nc.sbuf_tensor([_P, F], mybir.dt.float32))
        dt = ctx.enter_context(nc.sbuf_tensor([_P, F], mybir.dt.float32))
        sq = ctx.enter_context(nc.sbuf_tensor([_P, F], mybir.dt.float32))
        part = [
            ctx.enter_context(
                nc.sbuf_tensor(f"part{i}", [_P, 1], mybir.dt.float32)
            )
            for i in range(_K if _GATHER_MODE == "indirect4" else 1)
        ]
        red = ctx.enter_context(nc.sbuf_tensor([1, 1], mybir.dt.float32))
        psum = ctx.enter_context(nc.psum_tensor([1, 1], mybir.dt.float32))

        n_g = _K if _GATHER_MODE == "indirect4" else 1
        sem_g = [ctx.enter_context(nc.semaphore(f"sem_g{i}")) for i in range(n_g)]
        sem_l = ctx.enter_context(nc.semaphore("sem_l"))
        sem_x = ctx.enter_context(nc.semaphore("sem_x"))
        sem_v = ctx.enter_context(nc.semaphore("sem_v"))
        sem_m = ctx.enter_context(nc.semaphore("sem_m"))
        sem_r = ctx.enter_context(nc.semaphore("sem_r"))
        sem_d = ctx.enter_context(nc.semaphore("sem_d"))
        clearable = [sem_l, sem_x, *sem_g, sem_v, sem_m, sem_r]

        block = ctx.enter_context(nc.Block())

        @block.scalar
        def _(scalar):
            # Scalar's HWDGE ring: indices first (tiny, gates the gather),
            # x right behind it on the same FIFO ring.
            if _GATHER_MODE == "gather1":
                scalar.dma_start(out=lab[0:16, :], in_=idx[:]).then_inc(sem_l, 16)
            else:
                scalar.dma_start(out=lab[:], in_=idx[:]).then_inc(sem_l, 16)
            scalar.dma_start(
                out=xt[:], in_=x[:].rearrange("(p k) d -> p (k d)", p=_P)
            ).then_inc(sem_x, 16)
            # Park the store-ack wait here: the NEFF must not complete
            # before the output write is acked, and Scalar is idle.
            scalar.wait_ge(sem_d, 16)
            scalar.sem_clear(sem_d)

        @block.gpsimd
        def _(gpsimd):
            if _GATHER_MODE == "gather1":
                # The ucode reload overlaps the idx DMA flight.
                gpsimd.load_library(library_config.mlp)
                # dma_gather's index tile is addressed as [128, 32] but the
                # ucode only reads the 16 index channels in partitions 0-15
                # (the DMA above fills exactly those).
                gpsimd.wait_ge(sem_l, 16)
                gpsimd.dma_gather(
                    out_ap=gt[:].rearrange("p (q e) -> p q e", q=_K),
                    in_ap=centers[:],
                    idxs_ap=lab[:],
                    num_idxs=_ROWS,
                    num_idxs_reg=_ROWS,
                    elem_size=_D,
                ).then_inc(sem_g[0], 16)
            else:
                gpsimd.wait_ge(sem_l, 16)
                for k in range(_K):
                    gpsimd.indirect_dma_start(
                        out=gt[:, k * _D : (k + 1) * _D],
                        out_offset=None,
                        in_=centers[:],
                        in_offset=bass.IndirectOffsetOnAxis(
                            ap=lab[:, k : k + 1], axis=0
                        ),
                    ).then_inc(sem_g[k], 16)

        @block.vector
        def _(vector):
            vector.wait_ge(sem_x, 16)
            if _GATHER_MODE == "gather1":
                vector.wait_ge(sem_g[0], 16)
                vector.tensor_tensor(
                    out=dt[:], in0=xt[:], in1=gt[:], op=mybir.AluOpType.subtract
                )
                # sq = d*d, part = row-sum(sq), fused in one instruction.
                vector.scalar_tensor_tensor(
                    out=sq[:],
                    in0=dt[:],
                    scalar=0.0,
                    in1=dt[:],
                    op0=mybir.AluOpType.bypass,
                    op1=mybir.AluOpType.mult,
                    accum_out=part[0][:],
                )
            else:
                for k in range(_K):
                    blk = slice(k * _D, (k + 1) * _D)
                    vector.wait_ge(sem_g[k], 16)
                    vector.tensor_tensor(
                        out=dt[:, blk],
                        in0=xt[:, blk],
                        in1=gt[:, blk],
                        op=mybir.AluOpType.subtract,
                    )
                    vector.scalar_tensor_tensor(
                        out=sq[:, blk],
                        in0=dt[:, blk],
                        scalar=0.0,
                        in1=dt[:, blk],
                        op0=mybir.AluOpType.bypass,
                        op1=mybir.AluOpType.mult,
                        accum_out=part[k][:],
                    )
            # A reduce's output lands at instruction END; drain before
            # signaling so PE doesn't read a stale [128,1].
            vector.drain().then_inc(sem_v, 1)
            vector.wait_ge(sem_m, 1)
            vector.tensor_copy(out=red[:], in_=psum[:])
            vector.drain().then_inc(sem_r, 1)

        @block.tensor
        def _(tensor):
            tensor.wait_ge(sem_v, 1)
            n = len(part)
            for i, p_ in enumerate(part):
                mm = tensor.matmul(
                    psum[:], ones, p_[:], start=(i == 0), stop=(i == n - 1)
                )
                if i == n - 1:
                    mm.then_inc(sem_m, 1)

        @block.sync
        def _(sync):
            sync.wait_ge(sem_r, 1)
            sync.dma_start(out=out[:], in_=red[:]).then_inc(sem_d, 16)
            for s in clearable:
                sync.sem_clear(s)

    nc.compile()
    return nc


def _get_compiled():
    global _compiled
    if _compiled is None:
        _compiled = _build()
    return _compiled


def _host_idx(labels_core: np.ndarray) -> np.ndarray:
    """Per-core gather indices in the layout the device expects."""
    if _GATHER_MODE == "gather1":
        # dma_gather index i (= q*128 + p) fetches the center for x row
        # 4p + q; indices are wrapped into 16 partitions as
        # idx_tile[c, s] = U[s*16 + c].
        i = np.arange(_ROWS)
        u = labels_core[4 * (i % _P) + i // _P]
        return np.ascontiguousarray(
            u.reshape(_ROWS // 16, 16).T.astype(np.int16)
        )
    # indirect4: lab[p, k] = labels[4p + k], int32
    return np.ascontiguousarray(labels_core.reshape(_P, _K).astype(np.int32))


def _make_in_maps(x, labels_np, centers):
    return [
        {
            "x": np.ascontiguousarray(x[i * _ROWS : (i + 1) * _ROWS]),
            "idx": _host_idx(labels_np[i * _ROWS : (i + 1) * _ROWS]),
            "centers": centers,
        }
        for i in range(_N_CORES)
    ]


def kernel(x, labels, centers):
    from concourse.bass_utils import run_bass_kernel_spmd

    x = np.ascontiguousarray(np.asarray(x, dtype=np.float32))
    labels_np = np.asarray(labels).astype(np.int64)
    centers = np.ascontiguousarray(np.asarray(centers, dtype=np.float32))
    assert x.shape == (_B, _D) and labels_np.shape == (_B,)
    assert centers.shape == (_C, _D)

    nc = _get_compiled()
    in_maps = _make_in_maps(x, labels_np, centers)
    res = run_bass_kernel_spmd(nc, in_maps, list(range(_N_CORES)))

    # Host-side all-reduce of the per-core partials. Each row's squared
    # distance is hundreds for any non-degenerate input, so the per-element
    # clamp in the reference is a no-op on the selected entries; the (C-1)
    # masked-out zeros per row each clamp up to CLAMP_MIN.
    total = 0.0
    for i in range(_N_CORES):
        total += float(np.asarray(res.results[i]["out"], dtype=np.float64).sum())
    loss = total / _B + (_C - 1) * _CLAMP_MIN
    return np.asarray(loss, dtype=np.float32)
